# revision 1
# baseline (speedup 1.0000x reference)
"""CTC loss (warp-ctc semantics, size_average=True) on 8 Trainium2 NeuronCores.

Strategy (data-parallel over batch, 4 samples per core):
- Z[t,b] = sum_v exp(acts[t,b,v]) streamed as [128,8000] tiles; exp + free-dim
  sum fused in one ScalarE activation (accum_out). Host does log Z in float64.
- The alpha recursion runs in the LINEAR domain on unnormalized p~ = exp(acts
  at extended labels). States split into blank block (101) / label block (100),
  laid out [state-partition, (t,b)-free]. Using blank' = blank + shift(label)
  and label' = label + blank', each step is ONE TensorE matmul with a fixed
  shift stationary (no weight reloads; targets have no adjacent repeats) plus
  VectorE adds and the emission multiply; the [blank | label+blank] pre-add
  runs concurrently with the matmul so only add+mul sit on the serial chain.
  Every R steps the state-sum (ones-column matmuls) rescales alpha; factors
  are folded back in log-space on the host.
- The emission table (101 x T*8: gathered label acts + broadcast blank col)
  is host-prepared index prep; one DMA + one exp on device.
- Final: ll_b = log(alpha_fin) + sum log u - sum log Z  (host, float64);
  loss = -mean(ll).
"""

import sys
import types

import numpy as np

# ---- shim: provide antenv.axon_hooks (missing in this image) ----------------
_HOOK = [None]
try:
    import antenv.axon_hooks  # noqa: F401
except ImportError:
    try:
        from trn_agent_boot.trn_boot import _ntff_profile_via_ctypes

        _HOOK[0] = _ntff_profile_via_ctypes("/opt/axon/libaxon_pjrt.so")
    except Exception:
        pass
    _m = types.ModuleType("antenv.axon_hooks")
    _m.get_axon_ntff_profile_hook = lambda: _HOOK[0]
    _m.set_axon_ntff_profile_hook = lambda h: _HOOK.__setitem__(0, h)
    sys.modules["antenv.axon_hooks"] = _m
# -----------------------------------------------------------------------------

import concourse.bass as bass
import concourse.mybir as mybir
import concourse.tile as tile
from concourse.bass_utils import run_bass_kernel_spmd
from concourse.vector_clock import ScopedClock


# ---- walrus-compat patches: this walrus rejects Drains with >1 sem wait -----
def _my_drain_and_barrier(self, tick_clock, wait_clock):
    nc = self.nc
    dummy = nc.sync.nop(nofuse=True)
    wait_clock.add_sem_waits(dummy.ins, ScopedClock({None: tick_clock.global_clock}))
    si = dummy.ins.sync_info
    waits = list(si.on_wait) if si is not None else []
    if si is not None and len(waits) > 1:
        dummy.ins.sync_info = mybir.SyncInfo(
            on_wait=[waits[0]], on_update=list(si.on_update)
        )
        for w in waits[1:]:
            n = nc.sync.nop(nofuse=True)
            n.ins.sync_info = mybir.SyncInfo(on_wait=[w], on_update=[])
    nc.sync.drain()
    nc.all_engine_barrier()
    assert self.sems is not None
    popped = nc._tile_sem_poison_stack.pop()
    assert popped is self._sem_poison
    nc.clear_and_free_semaphores(list(self.sems.allocated().values()))
    nc.all_engine_barrier()


def _my_multi_engine_barrier(self, engines):
    # bare per-engine drains (this walrus rejects waits on Drain) followed by
    # an EVSEM sem-only all-engine barrier for the cross-engine sync.
    for e in engines:
        self.engines[e].drain()
    for inst in self._sem_only_all_engine_barrier_insts(f"aeb{self.next_id()}"):
        self.engines[inst.engine].add_instruction(inst)


tile.TileContext._drain_and_barrier = _my_drain_and_barrier
bass.Bass.multi_engine_barrier = _my_multi_engine_barrier


def _split_multiwait(nc):
    """This walrus build encodes at most one sync-wait per instruction; hoist
    extra waits onto preceding nofuse NOPs on the same engine."""
    n_new = 0
    for fn in nc.m.functions:
        for blk in fn.blocks:
            insts = blk.instructions
            i = 0
            while i < len(insts):
                ins = insts[i]
                si = getattr(ins, "sync_info", None)
                if si is not None and si.on_wait and len(si.on_wait) > 1:
                    waits = list(si.on_wait)
                    ins.sync_info = mybir.SyncInfo(
                        on_wait=[waits[-1]], on_update=list(si.on_update)
                    )
                    new_nops = []
                    for w in waits[:-1]:
                        nop = mybir.InstNoOp(
                            name=f"{ins.name}_wsplit{n_new}",
                            engine=ins.engine,
                            sync_info=mybir.SyncInfo(on_wait=[w], on_update=[]),
                            bass_nofuse=True,
                        )
                        n_new += 1
                        new_nops.append(nop)
                    insts[i:i] = new_nops
                    i += len(new_nops)
                i += 1
    return nc
# -----------------------------------------------------------------------------

T, B, V, L = 512, 32, 8000, 100
NCORES = 8
NB = B // NCORES  # 4 samples per core
W = 2 * NB  # alpha free width: cols 0..NB-1 blank block, NB..2NB-1 label block
NBLK = L + 1  # blank states
NLAB = L  # label states
RSC = 16  # rescale every RSC steps
F32 = mybir.dt.float32
I32 = mybir.dt.int32


def n_rescales(t_steps):
    return len([t for t in range(1, t_steps) if t % RSC == 0 and t != t_steps - 1])


def build_weights():
    """Static 0/1 lhsT weight matrices [K, M] for the per-step matmuls.

    psum[:, 0:NB]   = w_b0.T @ blank + w_n0.T @ label   (new blank block)
    psum[:, NB:2NB] = w_b1.T @ blank + w_n1.T @ label   (new label block)
    blank'[j] = blank[j] + label[j-1]; label'[j] = label[j] + blank[j] + label[j-1]
    """
    w_b0 = np.zeros((NBLK, NBLK), np.float32)
    w_n0 = np.zeros((NLAB, NBLK), np.float32)
    w_b1 = np.zeros((NBLK, NBLK), np.float32)
    w_n1 = np.zeros((NLAB, NBLK), np.float32)
    for k in range(NBLK):
        w_b0[k, k] = 1.0
        if k < NLAB:
            w_b1[k, k] = 1.0
    for k in range(NLAB):
        w_n0[k, k + 1] = 1.0
        w_n1[k, k] = 1.0
        if k + 1 < NLAB:
            w_n1[k, k + 1] = 1.0
    return w_b0, w_n0, w_b1, w_n1


def build_program(t_steps=T, split=True, do_stream=True, do_rec=True):
    """Build the per-core Bass program (identical for all cores)."""
    nc = bass.Bass("TRN2", target_bir_lowering=False, debug=False)
    ntile = NB * (t_steps // 128)
    nresc = n_rescales(t_steps)

    acts_d = nc.dram_tensor("acts", [NB * t_steps, V], F32, kind="ExternalInput")
    pg_d = nc.dram_tensor("pg", [NBLK, t_steps * W], F32, kind="ExternalInput")
    w_n0_d = nc.dram_tensor("w_n0", [NLAB, NBLK], F32, kind="ExternalInput")
    e0mask_d = nc.dram_tensor("e0mask", [NBLK, W], F32, kind="ExternalInput")

    zout_d = nc.dram_tensor("zout", [ntile, 128], F32, kind="ExternalOutput")
    afin_d = nc.dram_tensor("afin", [NBLK, W], F32, kind="ExternalOutput")
    ubuf_d = nc.dram_tensor("ubuf", [1, (nresc + 1) * W], F32, kind="ExternalOutput")

    with tile.TileContext(nc) as tc:
        with (
            tc.tile_pool(name="stream", bufs=2) as stream_pool,
            tc.tile_pool(name="escratch", bufs=1) as escratch_pool,
            tc.tile_pool(name="zpool", bufs=2) as zpool,
            tc.tile_pool(name="singles", bufs=1) as singles,
            tc.tile_pool(name="alpha", bufs=6) as alpha_pool,
            tc.tile_pool(name="mainpsum", bufs=4, space="PSUM") as mainpsum,
            tc.tile_pool(name="bpsum", bufs=2, space="PSUM") as bpsum,
            tc.tile_pool(name="upsum", bufs=2, space="PSUM") as upsum,
        ):
            # ---- static small inputs -> SBUF --------------------------------
            w_n0 = singles.tile([NLAB, NBLK], F32)
            e0mask = singles.tile([NBLK, W], F32)
            ones_row = singles.tile([1, NBLK], F32)  # lhsT for bcast [1]x[101]
            ones_colk = singles.tile([NBLK, 1], F32)  # lhsT for sums [101]x[1]
            nc.sync.dma_start(out=w_n0, in_=w_n0_d[:, :])
            nc.sync.dma_start(out=e0mask, in_=e0mask_d[:, :])
            nc.vector.memset(ones_row, 1.0)
            nc.vector.memset(ones_colk, 1.0)

            # ---- emission table p~ [state 0..100, (t, col)] -----------------
            # host supplies pg = raw acts at extended labels (blank cols are
            # the blank activation broadcast across state partitions).
            phat_raw = singles.tile([NBLK, t_steps * W], F32)
            phat = singles.tile([NBLK, t_steps * W], F32)
            nc.sync.dma_start(out=phat_raw, in_=pg_d[:, :])
            nc.scalar.activation(phat, phat_raw, mybir.ActivationFunctionType.Exp)

            # ---- streaming Z = sum_v exp(acts) ------------------------------
            for it in range(ntile if do_stream else 0):
                tile_a = stream_pool.tile([128, V], F32, tag="acts")
                nc.sync.dma_start(out=tile_a, in_=acts_d[it * 128 : (it + 1) * 128, :])
                e_t = escratch_pool.tile([128, V], F32, tag="escr")
                z_t = zpool.tile([128, 1], F32, tag="z")
                nc.scalar.activation(
                    e_t, tile_a, mybir.ActivationFunctionType.Exp, accum_out=z_t
                )
                nc.sync.dma_start(out=zout_d[it : it + 1, :], in_=z_t)

            # ---- alpha recursion -------------------------------------------
            ubuf = singles.tile([1, (nresc + 1) * W], F32)
            nc.vector.memset(ubuf, 1.0)

            alpha = alpha_pool.tile([NBLK, W], F32, tag="alpha")
            nc.vector.tensor_mul(alpha, phat[:, 0:W], e0mask)

            n_resc = 0
            for t in range(1, t_steps if do_rec else 1):
                # ps = shift(label) in both col blocks; stationary w_n0 is the
                # only per-step weight -> stays resident on the PE.
                ps = mainpsum.tile([NBLK, W], F32, tag="mps")
                lab_dup = bass.AP(
                    tensor=alpha.tensor,
                    offset=alpha[0:NLAB, NB:W].offset,
                    ap=[list(alpha[0:NLAB, NB:W].ap[0]), [0, 2], [1, NB]],
                )
                nc.tensor.matmul(ps, w_n0, lab_dup, start=True, stop=True)
                # yprep = [blank | label+blank] runs concurrently with the
                # matmul; the serial tail after PE is just add + mult.
                yprep = alpha_pool.tile([NBLK, W], F32, tag="yprep")
                nc.vector.tensor_copy(yprep[:, 0:NB], alpha[:, 0:NB])
                nc.vector.tensor_add(
                    yprep[:, NB:W], alpha[:, NB:W], alpha[:, 0:NB]
                )
                y = alpha_pool.tile([NBLK, W], F32, tag="yprep")
                nc.vector.tensor_add(y, yprep, ps[0:NBLK, :])
                alpha_next = alpha_pool.tile([NBLK, W], F32, tag="alpha")
                nc.vector.tensor_mul(
                    alpha_next, y, phat[:, t * W : (t + 1) * W]
                )
                alpha = alpha_next

                if t % RSC == 0 and t != t_steps - 1:
                    # u = sum_s alpha at partition 0 via ones-column matmuls
                    pu = upsum.tile([1, W], F32, tag="ups")
                    nc.tensor.matmul(
                        pu[:, 0:NB], ones_colk, alpha[0:NBLK, 0:NB], start=True, stop=False
                    )
                    nc.tensor.matmul(
                        pu[:, 0:NB],
                        ones_colk[0:NLAB, :],
                        alpha[0:NLAB, NB:W],
                        start=False,
                        stop=True,
                    )
                    nc.vector.tensor_copy(
                        ubuf[0:1, n_resc * W : n_resc * W + NB], pu[0:1, 0:NB]
                    )
                    rrec = singles.tile([1, NB], F32, tag="rrec")
                    nc.vector.reciprocal(rrec, pu[0:1, 0:NB])
                    pb = bpsum.tile([NBLK, W], F32, tag="rbc")
                    nc.tensor.matmul(pb[:, 0:NB], ones_row, rrec, start=True, stop=True)
                    nc.tensor.matmul(pb[:, NB:W], ones_row, rrec, start=True, stop=True)
                    alpha_r = alpha_pool.tile([NBLK, W], F32, tag="alpha")
                    nc.vector.tensor_mul(alpha_r, alpha, pb)
                    alpha = alpha_r
                    n_resc += 1

            nc.sync.dma_start(out=afin_d[:, :], in_=alpha)
            nc.sync.dma_start(out=ubuf_d[:, :], in_=ubuf)
    if split:
        _split_multiwait(nc)
    return nc


_NC_CACHE = {}


def _get_program(t_steps=T):
    if t_steps not in _NC_CACHE:
        _NC_CACHE[t_steps] = build_program(t_steps)
    return _NC_CACHE[t_steps]


def make_in_maps(acts, targets, t_steps=T):
    _, w_n0, _, _ = build_weights()
    e0mask = np.zeros((NBLK, W), np.float32)
    e0mask[0, :] = 1.0
    in_maps = []
    for c in range(NCORES):
        bs = slice(c * NB, (c + 1) * NB)
        acts_c = np.ascontiguousarray(
            acts[:t_steps, bs, :].transpose(1, 0, 2).reshape(NB * t_steps, V)
        )
        tg = targets[bs]  # [NB, L]
        a = acts[:t_steps, bs, :]  # [T, NB, V]
        pg = np.zeros((NBLK, t_steps, W), np.float32)
        # label cols: pg[l, t, NB+b] = a[t, b, tg[b, l]]
        gat = a[:, np.arange(NB)[:, None], tg]  # [NB, L] adv-idx -> [T, NB, L]
        pg[0:NLAB, :, NB : NB + NB] = gat.transpose(2, 0, 1)
        # blank cols: pg[:, t, b] = a[t, b, 0] broadcast over states
        pg[:, :, 0:NB] = a[:, :, 0][None, :, :]
        pg[NLAB:, :, NB:W] = -30.0
        pg = np.ascontiguousarray(pg.reshape(NBLK, t_steps * W))
        in_maps.append(
            {
                "acts": acts_c,
                "pg": pg,
                "w_n0": w_n0,
                "e0mask": e0mask,
            }
        )
    return in_maps


def finalize(results, t_steps=T):
    """Host-side combine: per-sample log-likelihoods -> scalar loss (f64)."""
    nresc = n_rescales(t_steps)
    ntchunk = t_steps // 128
    lls = []
    for c in range(NCORES):
        out = results[c]
        zout = out["zout"].astype(np.float64)  # [ntile, 128]
        afin = out["afin"].astype(np.float64)  # [NBLK, W]
        ubuf = out["ubuf"].astype(np.float64).reshape(-1, W)  # [nresc+1, W]
        for b in range(NB):
            logz = np.log(zout[b * ntchunk : (b + 1) * ntchunk, :]).sum()
            logu = np.log(ubuf[:nresc, b]).sum() if nresc else 0.0
            fin = afin[NBLK - 1, b] + afin[NLAB - 1, NB + b]
            lls.append(np.log(fin) + logu - logz)
    return -np.sum(lls) / B


def kernel(acts, targets, act_lens, label_lens):
    acts = np.asarray(acts, np.float32)
    targets = np.asarray(targets).astype(np.int64)
    act_lens = np.asarray(act_lens)
    label_lens = np.asarray(label_lens)
    assert acts.shape == (T, B, V), acts.shape
    assert targets.shape == (B, L)
    assert (act_lens == T).all() and (label_lens == L).all(), "only full lens supported"
    assert (targets[:, 1:] != targets[:, :-1]).all(), "adjacent repeats unsupported"

    nc = _get_program(T)
    in_maps = make_in_maps(acts, targets, T)
    res = run_bass_kernel_spmd(nc, in_maps, core_ids=list(range(NCORES)))
    return np.float32(finalize(res.results, T))


if __name__ == "__main__":
    rng = np.random.default_rng(0)
    acts = rng.standard_normal((T, B, V)).astype(np.float32)
    targets = rng.integers(1, V, (B, L)).astype(np.int32)
    for bb in range(B):
        while (targets[bb, 1:] == targets[bb, :-1]).any():
            targets[bb] = rng.integers(1, V, (L,)).astype(np.int32)
    act_lens = np.full(B, T, np.int32)
    label_lens = np.full(B, L, np.int32)
    out = kernel(acts, targets, act_lens, label_lens)
    print("kernel loss:", out)
    from ctc_numpy import ctc_ref_numpy

    ref = ctc_ref_numpy(acts, targets, act_lens, label_lens)
    print("ref    loss:", ref, " rel err:", abs(out - ref) / abs(ref))



# revision 2
# speedup vs baseline: 1.8318x; 1.8318x over previous
"""CTC loss (warp-ctc semantics, size_average=True) on 8 Trainium2 NeuronCores.

Strategy (data-parallel over batch, 4 samples per core):

- Z[t,b] = sum_v exp(acts[t,b,v]): acts staged to DRAM as bf16 (halves HBM
  traffic), streamed as [128, 8000] tiles; exp + free-dim sum fused in one
  ScalarE activation (accum_out). Host does log Z in float64.

- The alpha recursion runs in the LINEAR domain entirely on the Vector
  engine with states on the FREE axis: the 201 extended states are split
  into 13 chunks of 16 states, each chunk stored with a 16-cell left halo
  (W=32 cells/partition); 4 samples x 13 chunks + 3 spacer rows -> 64
  partitions. Each step is THREE in-order DVE tensor_tensor ops (no
  cross-engine syncs at all):
     c[1:]    = a[1:] + a[:-1]          (shift-add)
     c[3::2] += a[1:-2:2]               (skip-add, odd=label states only)
     a'       = c * phat[t]             (emission multiply)
  The halo goes stale by 2 cells/step; every 8 steps ONE stream_shuffle
  (also DVE) refreshes it from the left-neighbor partition. Spacer rows
  stay exactly zero (their emissions are 0), so chunk-0 halos read zeros.

- Range control without any device rescaling: the host folds a per-(t,b)
  constant (logmeanexp of the gathered emissions + 0.7788) into the
  emission table; measured cumulative drift stays within +-54 nats, well
  inside f32 range. Constants are added back exactly on the host.

- Final: ll_b = log(alpha_T[2L] + alpha_T[2L-1]) + sum_t c[t,b]
               - sum_t log Z[t,b]   (host, float64); loss = -mean(ll).
"""

import sys
import types

import numpy as np
import ml_dtypes

# ---- shim: provide antenv.axon_hooks (missing in this image) ----------------
_HOOK = [None]
try:
    import antenv.axon_hooks  # noqa: F401
except ImportError:
    try:
        from trn_agent_boot.trn_boot import _ntff_profile_via_ctypes

        _HOOK[0] = _ntff_profile_via_ctypes("/opt/axon/libaxon_pjrt.so")
    except Exception:
        pass
    _m = types.ModuleType("antenv.axon_hooks")
    _m.get_axon_ntff_profile_hook = lambda: _HOOK[0]
    _m.set_axon_ntff_profile_hook = lambda h: _HOOK.__setitem__(0, h)
    sys.modules["antenv.axon_hooks"] = _m
# -----------------------------------------------------------------------------

import concourse.bass as bass
import concourse.mybir as mybir
import concourse.tile as tile
from concourse.bass_utils import run_bass_kernel_spmd
from concourse.vector_clock import ScopedClock


# ---- walrus-compat patches: this walrus rejects Drains with >1 sem wait -----
def _my_drain_and_barrier(self, tick_clock, wait_clock):
    nc = self.nc
    dummy = nc.sync.nop(nofuse=True)
    wait_clock.add_sem_waits(dummy.ins, ScopedClock({None: tick_clock.global_clock}))
    si = dummy.ins.sync_info
    waits = list(si.on_wait) if si is not None else []
    if si is not None and len(waits) > 1:
        dummy.ins.sync_info = mybir.SyncInfo(
            on_wait=[waits[0]], on_update=list(si.on_update)
        )
        for w in waits[1:]:
            n = nc.sync.nop(nofuse=True)
            n.ins.sync_info = mybir.SyncInfo(on_wait=[w], on_update=[])
    nc.sync.drain()
    nc.all_engine_barrier()
    assert self.sems is not None
    popped = nc._tile_sem_poison_stack.pop()
    assert popped is self._sem_poison
    nc.clear_and_free_semaphores(list(self.sems.allocated().values()))
    nc.all_engine_barrier()


def _my_multi_engine_barrier(self, engines):
    # bare per-engine drains (this walrus rejects waits on Drain) followed by
    # an EVSEM sem-only all-engine barrier for the cross-engine sync.
    for e in engines:
        self.engines[e].drain()
    for inst in self._sem_only_all_engine_barrier_insts(f"aeb{self.next_id()}"):
        self.engines[inst.engine].add_instruction(inst)


tile.TileContext._drain_and_barrier = _my_drain_and_barrier
bass.Bass.multi_engine_barrier = _my_multi_engine_barrier


def _split_multiwait(nc):
    """This walrus build encodes at most one sync-wait per instruction; hoist
    extra waits onto preceding nofuse NOPs on the same engine."""
    n_new = 0
    for fn in nc.m.functions:
        for blk in fn.blocks:
            insts = blk.instructions
            i = 0
            while i < len(insts):
                ins = insts[i]
                si = getattr(ins, "sync_info", None)
                if si is not None and si.on_wait and len(si.on_wait) > 1:
                    waits = list(si.on_wait)
                    ins.sync_info = mybir.SyncInfo(
                        on_wait=[waits[-1]], on_update=list(si.on_update)
                    )
                    new_nops = []
                    for w in waits[:-1]:
                        nop = mybir.InstNoOp(
                            name=f"{ins.name}_wsplit{n_new}",
                            engine=ins.engine,
                            sync_info=mybir.SyncInfo(on_wait=[w], on_update=[]),
                            bass_nofuse=True,
                        )
                        n_new += 1
                        new_nops.append(nop)
                    insts[i:i] = new_nops
                    i += len(new_nops)
                i += 1
    return nc
# -----------------------------------------------------------------------------

T, B, V, L = 512, 32, 8000, 100
S = 2 * L + 1
NCORES = 8
NB = B // NCORES  # 4 samples per core
C, H, W = 16, 16, 32  # chunk states / halo / cells per partition
PCH = 13  # chunks per sample (13*16 = 208 >= 201)
NP = 64  # partitions: 2 quadrants x (13 + 3 spacer + 13 + 3 spacer)
EX = 8  # halo-exchange period (halo degrades 2 cells/step)
NTILE = NB * T // 128  # 16 streaming tiles
KCONST = 0.7788  # range-centering tilt (measured; see module docstring)
F32 = mybir.dt.float32
BF16 = mybir.dt.bfloat16

# within-quadrant partition roles: i%16 in [0,13) -> chunk; else spacer (zero)
SHUF_MASK = [
    (15 if i % 16 == 0 else (i if i % 16 >= PCH else i - 1)) for i in range(32)
]


def build_program(split=True):
    """Per-core Bass program (identical for all cores)."""
    nc = bass.Bass("TRN2", target_bir_lowering=False, debug=False)

    acts_d = nc.dram_tensor("acts", [NB * T, V], BF16, kind="ExternalInput")
    pg_d = nc.dram_tensor("pg", [NP, T * W], BF16, kind="ExternalInput")
    m0_d = nc.dram_tensor("m0", [NP, 2], F32, kind="ExternalInput")

    zout_d = nc.dram_tensor("zout", [128, NTILE], F32, kind="ExternalOutput")
    afin_d = nc.dram_tensor("afin", [NP, W], F32, kind="ExternalOutput")

    with tile.TileContext(nc) as tc:
        with (
            tc.tile_pool(name="singles", bufs=1) as singles,
            tc.tile_pool(name="stream", bufs=2) as stream_pool,
            tc.tile_pool(name="escr", bufs=2) as escr_pool,
            tc.tile_pool(name="alpha", bufs=2) as alpha_pool,
        ):
            # ---- small inputs + emission table ------------------------------
            pg_s = singles.tile([NP, T * W], BF16)
            nc.sync.dma_start(out=pg_s, in_=pg_d[:, :])
            m0 = singles.tile([NP, 2], F32)
            nc.sync.dma_start(out=m0, in_=m0_d[:, :])

            phat = singles.tile([NP, T * W], F32)
            nc.scalar.activation(phat, pg_s, mybir.ActivationFunctionType.Exp)

            zbuf = singles.tile([128, NTILE], F32)

            # ---- streaming Z = sum_v exp(acts) (DMA+ScalarE; overlaps DVE) --
            for it in range(NTILE):
                tile_a = stream_pool.tile([128, V], BF16, tag="acts")
                nc.sync.dma_start(
                    out=tile_a, in_=acts_d[it * 128 : (it + 1) * 128, :]
                )
                e_t = escr_pool.tile([128, V], BF16, tag="escr")
                nc.scalar.activation(
                    e_t,
                    tile_a,
                    mybir.ActivationFunctionType.Exp,
                    accum_out=zbuf[:, it : it + 1],
                )
            nc.sync.dma_start(out=zout_d[:, :], in_=zbuf)

            # ---- alpha recursion (all DVE, zero cross-engine syncs) ---------
            alpha = alpha_pool.tile([NP, W], F32, tag="alpha")
            nc.vector.memset(alpha, 0.0)
            nc.vector.tensor_mul(alpha[:, H : H + 2], phat[:, H : H + 2], m0)

            cs = singles.tile([NP, W], F32)
            nc.vector.memset(cs, 0.0)

            for t in range(1, T):
                nc.vector.tensor_add(cs[:, 1:W], alpha[:, 1:W], alpha[:, 0 : W - 1])
                nc.vector.tensor_add(
                    cs[:, 3:W:2], cs[:, 3:W:2], alpha[:, 1 : W - 2 : 2]
                )
                alpha_new = alpha_pool.tile([NP, W], F32, tag="alpha")
                nc.vector.tensor_mul(
                    alpha_new, cs, phat[:, t * W : (t + 1) * W]
                )
                alpha = alpha_new
                if t % EX == 0 and t != T - 1:
                    nc.vector.stream_shuffle(
                        alpha[:, 0:H], alpha[:, C : C + H], SHUF_MASK
                    )

            nc.sync.dma_start(out=afin_d[:, :], in_=alpha)
    if split:
        _split_multiwait(nc)
    return nc


_NC_CACHE = {}


def _get_program():
    if "nc" not in _NC_CACHE:
        _NC_CACHE["nc"] = build_program()
    return _NC_CACHE["nc"]


def _part_layout():
    """Per-partition (b_local, chunk) or None for spacer rows."""
    out = []
    for p in range(NP):
        i = p % 32
        j = i % 16
        out.append(
            None if j >= PCH else (2 * (p // 32) + (1 if i >= 16 else 0), j)
        )
    return out


def make_in_maps(acts, targets):
    """acts [T,B,V] f32, targets [B,L] int -> per-core input dicts + cc."""
    m0 = np.zeros((NP, 2), np.float32)
    for p in (0, 16, 32, 48):
        m0[p] = 1.0
    lay = _part_layout()

    in_maps = []
    ccs = []
    for core in range(NCORES):
        bs = slice(core * NB, (core + 1) * NB)
        acts_c = acts[:, bs, :]  # [T, NB, V]
        tg = targets[bs]  # [NB, L]

        ext = np.zeros((NB, S), np.int64)
        ext[:, 1::2] = tg
        gat = acts_c[:, np.arange(NB)[:, None], ext]  # [T, NB, S] f32
        gat64 = gat.astype(np.float64)
        cc = np.log(np.mean(np.exp(gat64), axis=2)) + KCONST  # [T, NB]
        pgv = gat64 - cc[:, :, None]  # [T, NB, S]

        pg = np.full((NP, T, W), -100.0, np.float64)
        for p, lo in enumerate(lay):
            if lo is None:
                continue
            b, ch = lo
            s0 = C * ch - H
            w_lo = max(0, -s0)
            w_hi = min(W, S - s0)
            if w_lo < w_hi:
                pg[p, :, w_lo:w_hi] = pgv[:, b, s0 + w_lo : s0 + w_hi]

        in_maps.append(
            {
                "acts": np.ascontiguousarray(
                    acts_c.transpose(1, 0, 2).reshape(NB * T, V)
                ).astype(ml_dtypes.bfloat16),
                "pg": np.ascontiguousarray(
                    pg.reshape(NP, T * W)
                ).astype(ml_dtypes.bfloat16),
                "m0": m0,
            }
        )
        ccs.append(cc)
    return in_maps, ccs


def finalize(results, ccs):
    """Host-side combine: per-sample log-likelihoods -> scalar loss (f64)."""
    lls = []
    for core in range(NCORES):
        out = results[core]
        zout = np.asarray(out["zout"], np.float64)  # [128, NTILE]
        afin = np.asarray(out["afin"], np.float64)  # [NP, W]
        cc = ccs[core]  # [T, NB]
        logz = np.log(zout)  # [128, NTILE]
        for b in range(NB):
            p = 32 * (b // 2) + 12 + 16 * (b % 2)  # last chunk's partition
            fin = afin[p, 23] + afin[p, 24]  # states 2L-1, 2L
            lz = logz[:, 4 * b : 4 * b + 4].sum()
            lls.append(np.log(fin) + cc[:, b].sum() - lz)
    return -np.sum(lls) / B


def kernel(acts, targets, act_lens, label_lens):
    acts = np.asarray(acts, np.float32)
    targets = np.asarray(targets).astype(np.int64)
    act_lens = np.asarray(act_lens)
    label_lens = np.asarray(label_lens)
    assert acts.shape == (T, B, V), acts.shape
    assert targets.shape == (B, L)
    assert (act_lens == T).all() and (label_lens == L).all(), "only full lens supported"
    assert (targets[:, 1:] != targets[:, :-1]).all(), "adjacent repeats unsupported"

    nc = _get_program()
    in_maps, ccs = make_in_maps(acts, targets)
    res = run_bass_kernel_spmd(nc, in_maps, core_ids=list(range(NCORES)))
    return np.float32(finalize(res.results, ccs))


if __name__ == "__main__":
    rng = np.random.default_rng(0)
    acts = rng.standard_normal((T, B, V)).astype(np.float32)
    targets = rng.integers(1, V, (B, L)).astype(np.int32)
    for bb in range(B):
        while (targets[bb, 1:] == targets[bb, :-1]).any():
            targets[bb] = rng.integers(1, V, (L,)).astype(np.int32)
    act_lens = np.full(B, T, np.int32)
    label_lens = np.full(B, L, np.int32)
    out = kernel(acts, targets, act_lens, label_lens)
    print("kernel loss:", out)
    from ctc_numpy import ctc_ref_numpy

    ref = ctc_ref_numpy(acts, targets, act_lens, label_lens)
    print("ref    loss:", ref, " rel err:", abs(out - ref) / abs(ref))


# revision 6
# speedup vs baseline: 1.9292x; 1.0532x over previous
"""CTC loss (warp-ctc semantics, size_average=True) on 8 Trainium2 NeuronCores.

Strategy (data-parallel over batch, 4 samples per core):

- Z[t,b] = sum_v exp(acts[t,b,v]): acts staged to DRAM as bf16 (halves HBM
  traffic), streamed as [128, 8000] tiles; exp + free-dim sum fused in one
  ScalarE activation (accum_out). Host does log Z in float64.

- The alpha recursion runs in the LINEAR domain entirely on the Vector
  engine with states on the FREE axis: the 201 extended states are split
  into 13 chunks of 16 states, each chunk stored with a 16-cell left halo
  (W=32 cells/partition); 4 samples x 13 chunks + 3 spacer rows -> 64
  partitions. Each step is THREE in-order DVE tensor_tensor ops (no
  cross-engine syncs at all):
     c[1:]    = a[1:] + a[:-1]          (shift-add)
     c[3::2] += a[1:-2:2]               (skip-add, odd=label states only)
     a'       = c * phat[t]             (emission multiply)
  The halo goes stale by 2 cells/step; every 8 steps ONE stream_shuffle
  (also DVE) refreshes it from the left-neighbor partition. Spacer rows
  stay exactly zero (their emissions are 0), so chunk-0 halos read zeros.

- Range control without any device rescaling: the host folds a per-(t,b)
  constant (logmeanexp of the gathered emissions + 0.7788) into the
  emission table; measured cumulative drift stays within +-54 nats, well
  inside f32 range. Constants are added back exactly on the host.

- Final: ll_b = log(alpha_T[2L] + alpha_T[2L-1]) + sum_t c[t,b]
               - sum_t log Z[t,b]   (host, float64); loss = -mean(ll).
"""

import sys
import types

import numpy as np
import ml_dtypes

# ---- shim: provide antenv.axon_hooks (missing in this image) ----------------
_HOOK = [None]
try:
    import antenv.axon_hooks  # noqa: F401
except ImportError:
    try:
        from trn_agent_boot.trn_boot import _ntff_profile_via_ctypes

        _HOOK[0] = _ntff_profile_via_ctypes("/opt/axon/libaxon_pjrt.so")
    except Exception:
        pass
    _m = types.ModuleType("antenv.axon_hooks")
    _m.get_axon_ntff_profile_hook = lambda: _HOOK[0]
    _m.set_axon_ntff_profile_hook = lambda h: _HOOK.__setitem__(0, h)
    sys.modules["antenv.axon_hooks"] = _m
# -----------------------------------------------------------------------------

import concourse.bass as bass
import concourse.mybir as mybir
import concourse.tile as tile
from concourse.bass_utils import run_bass_kernel_spmd
from concourse.vector_clock import ScopedClock


# ---- walrus-compat patches: this walrus rejects Drains with >1 sem wait -----
def _my_drain_and_barrier(self, tick_clock, wait_clock):
    nc = self.nc
    dummy = nc.sync.nop(nofuse=True)
    wait_clock.add_sem_waits(dummy.ins, ScopedClock({None: tick_clock.global_clock}))
    si = dummy.ins.sync_info
    waits = list(si.on_wait) if si is not None else []
    if si is not None and len(waits) > 1:
        dummy.ins.sync_info = mybir.SyncInfo(
            on_wait=[waits[0]], on_update=list(si.on_update)
        )
        for w in waits[1:]:
            n = nc.sync.nop(nofuse=True)
            n.ins.sync_info = mybir.SyncInfo(on_wait=[w], on_update=[])
    nc.sync.drain()
    nc.all_engine_barrier()
    assert self.sems is not None
    popped = nc._tile_sem_poison_stack.pop()
    assert popped is self._sem_poison
    nc.clear_and_free_semaphores(list(self.sems.allocated().values()))
    nc.all_engine_barrier()


def _my_multi_engine_barrier(self, engines):
    # bare per-engine drains (this walrus rejects waits on Drain) followed by
    # an EVSEM sem-only all-engine barrier for the cross-engine sync.
    for e in engines:
        self.engines[e].drain()
    for inst in self._sem_only_all_engine_barrier_insts(f"aeb{self.next_id()}"):
        self.engines[inst.engine].add_instruction(inst)


tile.TileContext._drain_and_barrier = _my_drain_and_barrier
bass.Bass.multi_engine_barrier = _my_multi_engine_barrier


def _split_multiwait(nc):
    """This walrus build encodes at most one sync-wait per instruction; hoist
    extra waits onto preceding nofuse NOPs on the same engine."""
    n_new = 0
    for fn in nc.m.functions:
        for blk in fn.blocks:
            insts = blk.instructions
            i = 0
            while i < len(insts):
                ins = insts[i]
                si = getattr(ins, "sync_info", None)
                if si is not None and si.on_wait and len(si.on_wait) > 1:
                    waits = list(si.on_wait)
                    ins.sync_info = mybir.SyncInfo(
                        on_wait=[waits[-1]], on_update=list(si.on_update)
                    )
                    new_nops = []
                    for w in waits[:-1]:
                        nop = mybir.InstNoOp(
                            name=f"{ins.name}_wsplit{n_new}",
                            engine=ins.engine,
                            sync_info=mybir.SyncInfo(on_wait=[w], on_update=[]),
                            bass_nofuse=True,
                        )
                        n_new += 1
                        new_nops.append(nop)
                    insts[i:i] = new_nops
                    i += len(new_nops)
                i += 1
    return nc
# -----------------------------------------------------------------------------

T, B, V, L = 512, 32, 8000, 100
S = 2 * L + 1
NCORES = 8
NB = B // NCORES  # 4 samples per core
C, H, W = 16, 16, 32  # chunk states / halo / cells per partition
PCH = 13  # chunks per sample (13*16 = 208 >= 201)
NP = 64  # partitions: 2 quadrants x (13 + 3 spacer + 13 + 3 spacer)
EX = 8  # halo-exchange period (halo degrades 2 cells/step)
NTILE = NB * T // 128  # 16 streaming tiles
KCONST = 0.7788  # range-centering tilt (measured; see module docstring)
F32 = mybir.dt.float32
BF16 = mybir.dt.bfloat16

# within-quadrant partition roles: i%16 in [0,13) -> chunk; else spacer (zero)
SHUF_MASK = [
    (15 if i % 16 == 0 else (i if i % 16 >= PCH else i - 1)) for i in range(32)
]


def _strip_same_engine_waits(nc, engines=(mybir.EngineType.DVE,)):
    """Drop sem waits that only order an engine against itself: the engine is
    in-order and its SBUF writes land before the engine frees (only the write
    ACK is deferred), so program order already guarantees RAW/WAR within the
    engine. The waits would otherwise serialize the pipelineable ACK half of
    every op (~58 DVE cycles each). Cross-engine waits are preserved."""
    own_sems = {e: set() for e in engines}
    for fn in nc.m.functions:
        for blk in fn.blocks:
            for ins in blk.instructions:
                if ins.engine in own_sems:
                    si = getattr(ins, "sync_info", None)
                    if si is not None:
                        for u in si.on_update:
                            if u.sync_type == "semaphore":
                                own_sems[ins.engine].add(u.id)
    n_strip = 0
    for fn in nc.m.functions:
        for blk in fn.blocks:
            for ins in blk.instructions:
                if ins.engine not in own_sems:
                    continue
                si = getattr(ins, "sync_info", None)
                if si is None or not si.on_wait:
                    continue
                keep = [
                    w
                    for w in si.on_wait
                    if not (
                        w.sync_type == "semaphore"
                        and w.id in own_sems[ins.engine]
                    )
                ]
                if len(keep) != len(si.on_wait):
                    n_strip += len(si.on_wait) - len(keep)
                    ins.sync_info = mybir.SyncInfo(
                        on_wait=keep, on_update=list(si.on_update)
                    )
    return n_strip


def build_program(split=True):
    """Per-core Bass program (identical for all cores)."""
    nc = bass.Bass("TRN2", target_bir_lowering=False, debug=False)

    acts_d = nc.dram_tensor("acts", [NB * T, V], BF16, kind="ExternalInput")
    pg_d = nc.dram_tensor("pg", [NP, T * W], BF16, kind="ExternalInput")
    m0_d = nc.dram_tensor("m0", [NP, 2], F32, kind="ExternalInput")

    zout_d = nc.dram_tensor("zout", [128, NTILE], F32, kind="ExternalOutput")
    afin_d = nc.dram_tensor("afin", [NP, W], F32, kind="ExternalOutput")

    with tile.TileContext(nc) as tc:
        with (
            tc.tile_pool(name="singles", bufs=1) as singles,
            tc.tile_pool(name="stream", bufs=2) as stream_pool,
            tc.tile_pool(name="escr", bufs=2) as escr_pool,
            tc.tile_pool(name="alpha", bufs=2) as alpha_pool,
        ):
            # ---- small inputs + emission table ------------------------------
            # pg upload + exp in 8 chunks so the recursion starts after the
            # first ~2us instead of waiting for the full table.
            m0 = singles.tile([NP, 2], F32)
            nc.sync.dma_start(out=m0, in_=m0_d[:, :])
            pg_s = singles.tile([NP, T * W], BF16)
            phat = singles.tile([NP, T * W], F32)
            NCHUNK = 8
            CH = T * W // NCHUNK
            for k in range(NCHUNK):
                sl = slice(k * CH, (k + 1) * CH)
                nc.sync.dma_start(out=pg_s[:, sl], in_=pg_d[:, sl])
                nc.scalar.activation(
                    phat[:, sl], pg_s[:, sl], mybir.ActivationFunctionType.Exp
                )

            zbuf = singles.tile([128, NTILE], F32)

            # ---- streaming Z = sum_v exp(acts) (DMA+ScalarE; overlaps DVE) --
            for it in range(NTILE):
                tile_a = stream_pool.tile([128, V], BF16, tag="acts")
                nc.sync.dma_start(
                    out=tile_a, in_=acts_d[it * 128 : (it + 1) * 128, :]
                )
                e_t = escr_pool.tile([128, V], BF16, tag="escr")
                nc.scalar.activation(
                    e_t,
                    tile_a,
                    mybir.ActivationFunctionType.Exp,
                    accum_out=zbuf[:, it : it + 1],
                )
            nc.sync.dma_start(out=zout_d[:, :], in_=zbuf)

            # ---- alpha recursion (all DVE, zero cross-engine syncs) ---------
            alpha = alpha_pool.tile([NP, W], F32, tag="alpha")
            nc.vector.memset(alpha, 0.0)
            nc.vector.tensor_mul(alpha[:, H : H + 2], phat[:, H : H + 2], m0)

            cs = singles.tile([NP, W], F32)
            nc.vector.memset(cs, 0.0)

            for t in range(1, T):
                nc.vector.tensor_add(cs[:, 1:W], alpha[:, 1:W], alpha[:, 0 : W - 1])
                nc.vector.tensor_add(
                    cs[:, 3:W:2], cs[:, 3:W:2], alpha[:, 1 : W - 2 : 2]
                )
                alpha_new = alpha_pool.tile([NP, W], F32, tag="alpha")
                nc.vector.tensor_mul(
                    alpha_new, cs, phat[:, t * W : (t + 1) * W]
                )
                alpha = alpha_new
                if t % EX == 0 and t != T - 1:
                    nc.vector.stream_shuffle(
                        alpha[:, 0:H], alpha[:, C : C + H], SHUF_MASK
                    )

            nc.sync.dma_start(out=afin_d[:, :], in_=alpha)
    import os
    if os.environ.get("CTC_STRIP", "0") == "1":
        _strip_same_engine_waits(nc)
    if split:
        _split_multiwait(nc)
    return nc


_NC_CACHE = {}


def _get_program():
    if "nc" not in _NC_CACHE:
        _NC_CACHE["nc"] = build_program()
    return _NC_CACHE["nc"]


def _part_layout():
    """Per-partition (b_local, chunk) or None for spacer rows."""
    out = []
    for p in range(NP):
        i = p % 32
        j = i % 16
        out.append(
            None if j >= PCH else (2 * (p // 32) + (1 if i >= 16 else 0), j)
        )
    return out


def make_in_maps(acts, targets):
    """acts [T,B,V] f32, targets [B,L] int -> per-core input dicts + cc."""
    m0 = np.zeros((NP, 2), np.float32)
    for p in (0, 16, 32, 48):
        m0[p] = 1.0
    lay = _part_layout()

    in_maps = []
    ccs = []
    for core in range(NCORES):
        bs = slice(core * NB, (core + 1) * NB)
        acts_c = acts[:, bs, :]  # [T, NB, V]
        tg = targets[bs]  # [NB, L]

        ext = np.zeros((NB, S), np.int64)
        ext[:, 1::2] = tg
        gat = acts_c[:, np.arange(NB)[:, None], ext]  # [T, NB, S] f32
        gat64 = gat.astype(np.float64)
        cc = np.log(np.mean(np.exp(gat64), axis=2)) + KCONST  # [T, NB]
        pgv = gat64 - cc[:, :, None]  # [T, NB, S]

        pg = np.full((NP, T, W), -100.0, np.float64)
        for p, lo in enumerate(lay):
            if lo is None:
                continue
            b, ch = lo
            s0 = C * ch - H
            w_lo = max(0, -s0)
            w_hi = min(W, S - s0)
            if w_lo < w_hi:
                pg[p, :, w_lo:w_hi] = pgv[:, b, s0 + w_lo : s0 + w_hi]

        in_maps.append(
            {
                "acts": np.ascontiguousarray(
                    acts_c.transpose(1, 0, 2).reshape(NB * T, V)
                ).astype(ml_dtypes.bfloat16),
                "pg": np.ascontiguousarray(
                    pg.reshape(NP, T * W)
                ).astype(ml_dtypes.bfloat16),
                "m0": m0,
            }
        )
        ccs.append(cc)
    return in_maps, ccs


def finalize(results, ccs):
    """Host-side combine: per-sample log-likelihoods -> scalar loss (f64)."""
    lls = []
    for core in range(NCORES):
        out = results[core]
        zout = np.asarray(out["zout"], np.float64)  # [128, NTILE]
        afin = np.asarray(out["afin"], np.float64)  # [NP, W]
        cc = ccs[core]  # [T, NB]
        logz = np.log(zout)  # [128, NTILE]
        for b in range(NB):
            p = 32 * (b // 2) + 12 + 16 * (b % 2)  # last chunk's partition
            fin = afin[p, 23] + afin[p, 24]  # states 2L-1, 2L
            lz = logz[:, 4 * b : 4 * b + 4].sum()
            lls.append(np.log(fin) + cc[:, b].sum() - lz)
    return -np.sum(lls) / B


def kernel(acts, targets, act_lens, label_lens):
    acts = np.asarray(acts, np.float32)
    targets = np.asarray(targets).astype(np.int64)
    act_lens = np.asarray(act_lens)
    label_lens = np.asarray(label_lens)
    assert acts.shape == (T, B, V), acts.shape
    assert targets.shape == (B, L)
    assert (act_lens == T).all() and (label_lens == L).all(), "only full lens supported"
    assert (targets[:, 1:] != targets[:, :-1]).all(), "adjacent repeats unsupported"

    nc = _get_program()
    in_maps, ccs = make_in_maps(acts, targets)
    res = run_bass_kernel_spmd(nc, in_maps, core_ids=list(range(NCORES)))
    return np.float32(finalize(res.results, ccs))


if __name__ == "__main__":
    rng = np.random.default_rng(0)
    acts = rng.standard_normal((T, B, V)).astype(np.float32)
    targets = rng.integers(1, V, (B, L)).astype(np.int32)
    for bb in range(B):
        while (targets[bb, 1:] == targets[bb, :-1]).any():
            targets[bb] = rng.integers(1, V, (L,)).astype(np.int32)
    act_lens = np.full(B, T, np.int32)
    label_lens = np.full(B, L, np.int32)
    out = kernel(acts, targets, act_lens, label_lens)
    print("kernel loss:", out)
    from ctc_numpy import ctc_ref_numpy

    ref = ctc_ref_numpy(acts, targets, act_lens, label_lens)
    print("ref    loss:", ref, " rel err:", abs(out - ref) / abs(ref))
